# revision 3
# baseline (speedup 1.0000x reference)
"""Trainium2 Bass kernel for nn_BinaryDecorator (binarized linear layer).

Computes, for x:[8192,4096] f32, W:[4096,4096] f32 (values are +-1), b:[4096]:
    x_mean = mean(|x|)                     (scalar)
    out    = (sign(x) @ sign(W)^T + b) * x_mean     -> [8192, 4096] f32

Distribution: 4 token-groups x 2 out-groups across 8 NeuronCores.
Per core: x_sh [2048,4096], w_sh [2048,4096], b_sh [1,2048] -> out_sh [2048,2048].

Key ideas:
  - +-1 (and +-0.5) values are EXACT in fp8e4 -> run the 275 GFLOP matmul in
    fp8 with DoubleRow perf mode (2 fp8 MACs/cell/cycle), fp32 PSUM accumulate
    is exact integers -> bit-accurate math at ~2x bf16 speed.
  - sign(x) computed on DVE as (x>=0)->{1,0} then -0.5 -> {+0.5,-0.5} in one
    tensor_scalar op; the extra factor 2 is folded into the output scale.
  - Both matmul operands need K (in_features) on partitions; transposes are
    done on the PE as regular matmuls against an fp8 identity (HAM-warm,
    ~matmul rate), evacuated PSUM->SBUF by the Scalar engine.
  - mean|x|: each core reduces a disjoint 1/8 of x (extra xr input), 8-core
    AllReduce of the scalar (runs on TOPSP/SDMA, overlapped), then the scale
    rides the PSUM->SBUF eviction for free via ScalarE activation(scale=AP).
"""

import sys

if "/opt/trn_rl_repo" not in sys.path:
    sys.path.insert(0, "/opt/trn_rl_repo")

import numpy as np

N_CORES = 8
TG, OG = 4, 2  # token groups x out-feature groups
N_TOK, D_IN, D_OUT = 8192, 4096, 4096
M = N_TOK // TG      # 2048 tokens per core
N = D_OUT // OG      # 2048 out features per core
K = D_IN             # 4096 contraction
P = 128
KS = K // P          # 32 k-subtiles
MB = M // P          # 16 token blocks
WB = N // P          # 16 W row blocks
FREE = 512
NCOL = N // FREE     # 4 psum columns
RED_ROWS = N_TOK // N_CORES  # 1024 rows reduced per core for mean|x|
TOTAL_X = float(N_TOK * D_IN)  # 2^25

_cache = {}


def _build():
    import concourse.bass as bass  # noqa: F401
    import concourse.mybir as mybir
    from concourse import bacc, tile
    from concourse.masks import make_identity
    import concourse.bass_isa as bass_isa
    from contextlib import ExitStack

    F32 = mybir.dt.float32
    FP8 = mybir.dt.float8e4
    AF = mybir.ActivationFunctionType
    ALU = mybir.AluOpType

    nc = bacc.Bacc(
        "TRN2",
        target_bir_lowering=False,
        debug=False,
        enable_asserts=False,
        num_devices=N_CORES,
    )

    x = nc.dram_tensor("x_sh", [M, K], F32, kind="ExternalInput")
    w = nc.dram_tensor("w_sh", [N, K], F32, kind="ExternalInput")
    b = nc.dram_tensor("b_sh", [1, N], F32, kind="ExternalInput")
    xr = nc.dram_tensor("xr_sh", [RED_ROWS, K], F32, kind="ExternalInput")
    out = nc.dram_tensor("out_sh", [M, N], F32, kind="ExternalOutput")

    with tile.TileContext(nc) as tc, ExitStack() as ctx:
        const = ctx.enter_context(tc.tile_pool(name="const", bufs=1))
        wbt_pool = ctx.enter_context(tc.tile_pool(name="wbt", bufs=1))
        xbt_pool = ctx.enter_context(tc.tile_pool(name="xbt", bufs=3))
        xstage_pool = ctx.enter_context(tc.tile_pool(name="xstage", bufs=3))
        xsign_pool = ctx.enter_context(tc.tile_pool(name="xsign", bufs=3))
        wstage_pool = ctx.enter_context(tc.tile_pool(name="wstage", bufs=3))
        rstage_pool = ctx.enter_context(tc.tile_pool(name="rstage", bufs=2))
        ostage_pool = ctx.enter_context(tc.tile_pool(name="ostage", bufs=2))
        stat_pool = ctx.enter_context(tc.tile_pool(name="stats", bufs=1))
        tpsum_pool = ctx.enter_context(tc.tile_pool(name="tpsum", bufs=3, space="PSUM"))
        mpsum_pool = ctx.enter_context(tc.tile_pool(name="mpsum", bufs=1, space="PSUM"))
        dram = ctx.enter_context(tc.tile_pool(name="dram", bufs=1, space="DRAM"))

        # ---- constants ----
        ident = const.tile([P, P], FP8)
        make_identity(nc, ident)

        b_row = const.tile([1, N], F32)
        nc.sync.dma_start(b_row[:], b[:, :])
        Bb = const.tile([P, N], F32)
        nc.gpsimd.partition_broadcast(Bb[:], b_row[:])

        # ---- W prep: wbT [128, 32ksub, 2048 outs] fp8 (values +-1, exact) ----
        # DMA-cast f32->fp8 during load (SWDGE), then PE transpose via identity
        # matmul, ScalarE evacuates PSUM->SBUF.
        wbT = wbt_pool.tile([P, KS, N], FP8)
        for wb_i in range(WB):
            wt = wstage_pool.tile([P, K], FP8, tag="wstage")
            nc.gpsimd.dma_start(wt[:], w[wb_i * P:(wb_i + 1) * P, :])
            for jg in range(KS // 4):
                pt = tpsum_pool.tile([P, FREE], F32, tag="tp")
                for q in range(4):
                    j = jg * 4 + q
                    nc.tensor.matmul(
                        pt[:, q * P:(q + 1) * P],
                        lhsT=wt[:, j * P:(j + 1) * P],
                        rhs=ident[:],
                        start=True,
                        stop=True,
                    )
                nc.scalar.activation(
                    wbT[:, jg * 4:jg * 4 + 4, wb_i * P:(wb_i + 1) * P],
                    pt.rearrange("p (a t) -> p a t", a=4),
                    AF.Copy,
                )

        # ---- mean|x| over this core's disjoint 1/8 of x ----
        n_red = RED_ROWS // P  # 8 row-blocks, each reduced in 2 halves
        stats = stat_pool.tile([P, 2 * n_red], F32)
        for i in range(n_red):
            for h in range(2):
                rt = rstage_pool.tile([P, K // 2], F32, tag="rstage")
                nc.sync.dma_start(
                    rt[:], xr[i * P:(i + 1) * P, h * (K // 2):(h + 1) * (K // 2)]
                )
                nc.vector.tensor_reduce(
                    stats[:, 2 * i + h:2 * i + h + 1],
                    rt[:],
                    axis=mybir.AxisListType.X,
                    op=ALU.add,
                    apply_absolute_value=True,
                )
        ssum = stat_pool.tile([P, 1], F32)
        nc.vector.tensor_reduce(
            ssum[:], stats[:], axis=mybir.AxisListType.X, op=ALU.add
        )
        gsum = stat_pool.tile([P, 1], F32)
        nc.gpsimd.partition_all_reduce(
            gsum[:], ssum[:], channels=P, reduce_op=bass_isa.ReduceOp.add
        )

        cc_in = dram.tile([1, 8], F32)
        cc_out = dram.tile([1, 8], F32, addr_space="Shared")
        nc.sync.dma_start(cc_in[0:1, 0:1], gsum[0:1, 0:1])
        nc.gpsimd.collective_compute(
            "AllReduce",
            ALU.add,
            replica_groups=[list(range(N_CORES))],
            ins=[cc_in.opt()],
            outs=[cc_out.opt()],
        )
        xmt = stat_pool.tile([P, 1], F32)
        nc.vector.memset(xmt[:], 0.0)
        nc.sync.dma_start(xmt[0:1, 0:1], cc_out[0:1, 0:1])
        gbc = stat_pool.tile([P, 1], F32)
        nc.gpsimd.partition_broadcast(gbc[:], xmt[0:1, :])

        # psum holds S/2 (xb=+-0.5); out = psum * (G * 2/2^25) + b * (G/2^25)
        scale_mm = stat_pool.tile([P, 1], F32)
        nc.vector.tensor_scalar_mul(scale_mm[:], gbc[:], float(2.0 / TOTAL_X))
        Bbs = const.tile([P, N], F32)
        nc.vector.tensor_scalar(
            Bbs[:], Bb[:], gbc[:], float(1.0 / TOTAL_X), ALU.mult, ALU.mult
        )

        # ---- main loop over token blocks ----
        for m in range(MB):
            # x prep: sign -> fp8 (+-0.5), PE transpose into xbt [128, 32, 128]
            xbt = xbt_pool.tile([P, KS, P], FP8, tag="xbt")
            for h in range(2):
                xt = xstage_pool.tile([P, K // 2], F32, tag="xstage")
                nc.sync.dma_start(
                    xt[:], x[m * P:(m + 1) * P, h * (K // 2):(h + 1) * (K // 2)]
                )
                xs = xsign_pool.tile([P, K // 2], FP8, tag="xsign")
                nc.vector.tensor_scalar(
                    xs[:], xt[:], 0.0, 0.5, ALU.is_ge, ALU.subtract
                )
                for jg in range(KS // 8):  # 4 groups of 4 ksubs per half
                    pt = tpsum_pool.tile([P, FREE], F32, tag="tp")
                    for q in range(4):
                        j = jg * 4 + q  # ksub within this half
                        nc.tensor.matmul(
                            pt[:, q * P:(q + 1) * P],
                            lhsT=xs[:, j * P:(j + 1) * P],
                            rhs=ident[:],
                            start=True,
                            stop=True,
                        )
                    ks0 = h * (KS // 2) + jg * 4
                    nc.scalar.activation(
                        xbt[:, ks0:ks0 + 4, :],
                        pt.rearrange("p (a t) -> p a t", a=4),
                        AF.Copy,
                    )

            # fp8 DoubleRow matmul chain: 4 psum banks over 16 k-pairs
            pbanks = [
                mpsum_pool.tile([P, FREE], F32, tag=f"mp{i}", name=f"mp{i}_{m}")
                for i in range(NCOL)
            ]
            for kj in range(KS // 2):
                lhsT = xbt[:, 2 * kj:2 * kj + 2, :]
                for ncol in range(NCOL):
                    nc.tensor.matmul(
                        pbanks[ncol],
                        lhsT,
                        wbT[:, 2 * kj:2 * kj + 2, ncol * FREE:(ncol + 1) * FREE],
                        start=(kj == 0),
                        stop=(kj == KS // 2 - 1),
                        perf_mode=mybir.MatmulPerfMode.DoubleRow,
                    )

            # epilogue: scale by x_mean on the PSUM->SBUF evict, add scaled bias
            ot = ostage_pool.tile([P, N], F32, tag="ostage")
            for ncol in range(NCOL):
                nc.scalar.activation(
                    ot[:, ncol * FREE:(ncol + 1) * FREE],
                    pbanks[ncol],
                    AF.Copy,
                    bias=0.0,
                    scale=scale_mm[:],
                )
            nc.vector.tensor_tensor(ot[:], ot[:], Bbs[:], ALU.add)
            nc.sync.dma_start(out[m * P:(m + 1) * P, :], ot[:])

    nc.compile()
    return nc


def _get_nc():
    if "nc" not in _cache:
        _cache["nc"] = _build()
    return _cache["nc"]


def _make_in_maps(x, W, b):
    x = np.ascontiguousarray(x, dtype=np.float32)
    W = np.ascontiguousarray(W, dtype=np.float32)
    b = np.ascontiguousarray(b, dtype=np.float32)
    in_maps = []
    for c in range(N_CORES):
        tg, og = c // OG, c % OG
        in_maps.append(
            {
                "x_sh": np.ascontiguousarray(x[tg * M:(tg + 1) * M]),
                "w_sh": np.ascontiguousarray(W[og * N:(og + 1) * N]),
                "b_sh": np.ascontiguousarray(b[og * N:(og + 1) * N]).reshape(1, N),
                "xr_sh": np.ascontiguousarray(x[c * RED_ROWS:(c + 1) * RED_ROWS]),
            }
        )
    return in_maps


def _run(x, W, b, trace=False):
    from concourse.bass_utils import run_bass_kernel_spmd

    nc = _get_nc()
    res = run_bass_kernel_spmd(
        nc, _make_in_maps(x, W, b), core_ids=list(range(N_CORES)), trace=trace
    )
    full = np.empty((N_TOK, D_OUT), dtype=np.float32)
    for c, r in enumerate(res.results):
        tg, og = c // OG, c % OG
        full[tg * M:(tg + 1) * M, og * N:(og + 1) * N] = r["out_sh"]
    return full, res


def kernel(x, W, b):
    full, _ = _run(x, W, b, trace=False)
    return full
